# revision 4
# baseline (speedup 1.0000x reference)
"""Pairwise cosine similarity [8192,1024]x[8192,1024] -> [8192,8192] on 8 trn2 cores.

Sharding: 4x2 grid. Core (i,j) takes input1 rows [2048*i, 2048*(i+1)) and
input2 rows [4096*j, 4096*(j+1)), computes its [2048, 4096] output block.
All cores run one SPMD program; the host slices inputs and assembles blocks.

Device program (per core):
  1. Normalize rows of x and y on-chip: ACT square w/ accum_out -> sqrt ->
     max(eps) -> reciprocal -> ACT copy w/ per-partition scale.
  2. PE transpose-mode (exact for fp32) moves D onto partitions:
     x^T [128, 8k, 2048], y^T chunks [128, 8k, 512].
  3. fp32r matmuls (1 cyc/row at N=512) accumulate 8 K-slabs into PSUM;
     DVE/ACT copy PSUM->SBUF; DMA out.
"""

import numpy as np

import concourse.bacc as bacc
import concourse.bass as bass
import concourse.masks as masks
import concourse.mybir as mybir
import concourse.tile as tile
from concourse.bass_utils import run_bass_kernel_spmd

P = 128
D = 1024
KD = D // P  # 8 k-slabs of the contraction dim
N_FULL = 8192
M_FULL = 8192
GRID_N, GRID_M = 4, 2
N_LOC = N_FULL // GRID_N  # 2048
M_LOC = M_FULL // GRID_M  # 4096
EPS = 1e-8
F32 = mybir.dt.float32
F32R = mybir.dt.float32r

# Set by test harness to capture profiling info; harness-default is off.
TRACE = False
LAST_RESULT = None


def build(n_loc=N_LOC, m_loc=M_LOC, n_cores=8):
    """Build + compile the SPMD program for one core's [n_loc, m_loc] block."""
    nt_tiles = n_loc // P
    yt_tiles = m_loc // P
    mc_chunks = m_loc // 512

    nc = bacc.Bacc("TRN2", target_bir_lowering=False, debug=False,
                   num_devices=n_cores)
    x_d = nc.dram_tensor("x", [n_loc, D], F32, kind="ExternalInput").ap()
    y_d = nc.dram_tensor("y", [m_loc, D], F32, kind="ExternalInput").ap()
    o_d = nc.dram_tensor("o", [n_loc, m_loc], F32, kind="ExternalOutput").ap()

    with tile.TileContext(nc) as tc:
        with (
            tc.tile_pool(name="persist", bufs=1) as persist,
            tc.tile_pool(name="stage", bufs=4) as stage,
            tc.tile_pool(name="sq", bufs=2) as sqp,
            tc.tile_pool(name="small", bufs=4) as small,
            tc.tile_pool(name="ytp", bufs=2) as ytp,
            tc.tile_pool(name="outp", bufs=4) as outp,
            tc.tile_pool(name="pst", bufs=2, space=bass.MemorySpace.PSUM) as pst,
            tc.tile_pool(name="pso", bufs=4, space=bass.MemorySpace.PSUM) as pso,
        ):
            ident = persist.tile([P, P], F32)
            masks.make_identity(nc, ident[:])
            # f32r tiles: the PSUM->SBUF copy rounds to fp32r, which the
            # walrus verifier requires for fp32r matmul operands.
            xt_sb = persist.tile([P, KD, n_loc], F32R)

            def norm_transpose(src_rows, dst, dst_col0):
                # One [128, D] tile: load, normalize rows, transpose the 8
                # [128,128] k-subtiles into dst[:, k, dst_col0:dst_col0+128].
                ts = stage.tile([P, D], F32)
                nc.sync.dma_start(ts[:], src_rows)
                sq = sqp.tile([P, D], F32)
                ss = small.tile([P, 1], F32)
                nc.scalar.activation(sq[:], ts[:],
                                     mybir.ActivationFunctionType.Square,
                                     accum_out=ss[:])
                nrm = small.tile([P, 1], F32)
                nc.scalar.sqrt(nrm[:], ss[:])
                nc.vector.tensor_scalar_max(nrm[:], nrm[:], EPS)
                rinv = small.tile([P, 1], F32)
                nc.vector.reciprocal(rinv[:], nrm[:])
                nc.scalar.activation(ts[:], ts[:],
                                     mybir.ActivationFunctionType.Copy,
                                     scale=rinv[:])
                for kg in range(KD // 4):
                    ps = pst.tile([P, 4, P], F32)
                    for kk in range(4):
                        k = kg * 4 + kk
                        nc.tensor.transpose(ps[:, kk, :],
                                            ts[:, k * P:(k + 1) * P],
                                            ident[:])
                    nc.vector.tensor_copy(
                        dst[:, kg * 4:(kg + 1) * 4, dst_col0:dst_col0 + P],
                        ps[:])

            for xt in range(nt_tiles):
                norm_transpose(x_d[xt * P:(xt + 1) * P, :], xt_sb, xt * P)

            for mc in range(mc_chunks):
                yt_sb = ytp.tile([P, KD, 512], F32R)
                for v in range(4):
                    yt = mc * 4 + v
                    norm_transpose(y_d[yt * P:(yt + 1) * P, :], yt_sb, v * P)
                for nt in range(nt_tiles):
                    po = pso.tile([P, 512], F32)
                    for k in range(KD):
                        nc.tensor.matmul(
                            po[:],
                            xt_sb[:, k, nt * P:(nt + 1) * P],
                            yt_sb[:, k, :],
                            start=(k == 0),
                            stop=(k == KD - 1))
                    ot = outp.tile([P, 512], F32)
                    if nt % 2 == 0:
                        nc.vector.tensor_copy(ot[:], po[:])
                    else:
                        nc.scalar.copy(ot[:], po[:])
                    nc.sync.dma_start(
                        o_d[nt * P:(nt + 1) * P, mc * 512:(mc + 1) * 512],
                        ot[:])

    nc.compile()
    return nc


_NC = None


def _get_nc():
    global _NC
    if _NC is None:
        _NC = build()
    return _NC


def kernel(input1, input2):
    global LAST_RESULT
    input1 = np.ascontiguousarray(np.asarray(input1, dtype=np.float32))
    input2 = np.ascontiguousarray(np.asarray(input2, dtype=np.float32))
    nc = _get_nc()
    in_maps = []
    for i in range(GRID_N):
        for j in range(GRID_M):
            in_maps.append({
                "x": input1[i * N_LOC:(i + 1) * N_LOC],
                "y": input2[j * M_LOC:(j + 1) * M_LOC],
            })
    res = run_bass_kernel_spmd(nc, in_maps, list(range(GRID_N * GRID_M)),
                               trace=TRACE)
    LAST_RESULT = res
    out = np.empty((N_FULL, M_FULL), dtype=np.float32)
    idx = 0
    for i in range(GRID_N):
        for j in range(GRID_M):
            out[i * N_LOC:(i + 1) * N_LOC,
                j * M_LOC:(j + 1) * M_LOC] = res.results[idx]["o"]
            idx += 1
    return out


# revision 8
# speedup vs baseline: 1.1220x; 1.1220x over previous
"""Pairwise cosine similarity [8192,1024]x[8192,1024] -> [8192,8192] on 8 trn2 cores.

Sharding: 4x2 grid. Core (i,j) takes input1 rows [2048*i, 2048*(i+1)) and
input2 rows [4096*j, 4096*(j+1)), computes its [2048, 4096] output block.
All cores run one SPMD program; the host slices inputs and assembles blocks.

Device program (per core):
  1. Normalize rows of x and y on-chip: ACT square w/ accum_out -> sqrt ->
     max(eps) -> reciprocal -> ACT copy w/ per-partition scale.
  2. PE transpose-mode (exact for fp32) moves D onto partitions:
     x^T [128, 8k, 2048], y^T chunks [128, 8k, 512].
  3. fp32r matmuls (1 cyc/row at N=512) accumulate 8 K-slabs into PSUM;
     DVE/ACT copy PSUM->SBUF; DMA out.
"""

import numpy as np

import concourse.bacc as bacc
import concourse.bass as bass
import concourse.masks as masks
import concourse.mybir as mybir
import concourse.tile as tile
from concourse.bass_utils import run_bass_kernel_spmd

P = 128
D = 1024
KD = D // P  # 8 k-slabs of the contraction dim
N_FULL = 8192
M_FULL = 8192
GRID_N, GRID_M = 4, 2
N_LOC = N_FULL // GRID_N  # 2048
M_LOC = M_FULL // GRID_M  # 4096
EPS = 1e-8
F32 = mybir.dt.float32
F32R = mybir.dt.float32r

# Set by test harness to capture profiling info; harness-default is off.
TRACE = False
LAST_RESULT = None


def build(n_loc=N_LOC, m_loc=M_LOC, n_cores=8):
    """Build + compile the SPMD program for one core's [n_loc, m_loc] block."""
    nt_tiles = n_loc // P
    yt_tiles = m_loc // P
    mc_chunks = m_loc // 512

    nc = bacc.Bacc("TRN2", target_bir_lowering=False, debug=False,
                   num_devices=n_cores)
    x_d = nc.dram_tensor("x", [n_loc, D], F32, kind="ExternalInput").ap()
    y_d = nc.dram_tensor("y", [m_loc, D], F32, kind="ExternalInput").ap()
    o_d = nc.dram_tensor("o", [n_loc, m_loc], F32, kind="ExternalOutput").ap()

    with tile.TileContext(nc) as tc:
        with (
            tc.tile_pool(name="persist", bufs=1) as persist,
            tc.tile_pool(name="stage", bufs=4) as stage,
            tc.tile_pool(name="sq", bufs=2) as sqp,
            tc.tile_pool(name="small", bufs=4) as small,
            tc.tile_pool(name="ytp", bufs=2) as ytp,
            tc.tile_pool(name="outp", bufs=4) as outp,
            tc.tile_pool(name="pst", bufs=2, space=bass.MemorySpace.PSUM) as pst,
            tc.tile_pool(name="pso", bufs=4, space=bass.MemorySpace.PSUM) as pso,
        ):
            ident = persist.tile([P, P], F32)
            masks.make_identity(nc, ident[:])
            # f32r identity so transposes run as fp32r (1.5 vs 2 cyc/row);
            # 0.0/1.0 are exact in fp32r so the transpose stays exact.
            ident_r = persist.tile([P, P], F32R)
            nc.vector.tensor_copy(ident_r[:], ident[:])
            # Per-n-tile x^T tiles (separate tags -> granular deps so the
            # first matmuls start before the whole X phase finishes).
            xts = [persist.tile([P, KD, P], F32R, name=f"xt{i}", tag=f"xt{i}")
                   for i in range(nt_tiles)]

            def norm_transpose(src_rows, dst, dst_col0):
                # One [128, D] tile: load, normalize rows (rounding to f32r),
                # transpose the 8 [128,128] k-subtiles into
                # dst[:, k, dst_col0:dst_col0+128].
                ts = stage.tile([P, D], F32)
                nc.sync.dma_start(ts[:], src_rows)
                sq = sqp.tile([P, D], F32)
                ss = small.tile([P, 1], F32)
                nc.scalar.activation(sq[:], ts[:],
                                     mybir.ActivationFunctionType.Square,
                                     accum_out=ss[:])
                nrm = small.tile([P, 1], F32)
                nc.scalar.sqrt(nrm[:], ss[:])
                nc.vector.tensor_scalar_max(nrm[:], nrm[:], EPS)
                rinv = small.tile([P, 1], F32)
                nc.vector.reciprocal(rinv[:], nrm[:])
                tsr = sqp.tile([P, D], F32R, name="tsr", tag="tsr")
                nc.scalar.activation(tsr[:], ts[:],
                                     mybir.ActivationFunctionType.Copy,
                                     scale=rinv[:])
                for kg in range(KD // 4):
                    ps = pst.tile([P, 4, P], F32R)
                    for kk in range(4):
                        k = kg * 4 + kk
                        nc.tensor.transpose(ps[:, kk, :],
                                            tsr[:, k * P:(k + 1) * P],
                                            ident_r[:])
                    nc.vector.tensor_copy(
                        dst[:, kg * 4:(kg + 1) * 4, dst_col0:dst_col0 + P],
                        ps[:])

            for mc in range(mc_chunks):
                yt_sb = ytp.tile([P, KD, 512], F32R)
                for v in range(4):
                    yt = mc * 4 + v
                    norm_transpose(y_d[yt * P:(yt + 1) * P, :], yt_sb, v * P)
                if mc == 0:
                    for xt in range(nt_tiles):
                        norm_transpose(x_d[xt * P:(xt + 1) * P, :], xts[xt], 0)
                for nt in range(nt_tiles):
                    po = pso.tile([P, 512], F32)
                    for k in range(KD):
                        nc.tensor.matmul(
                            po[:],
                            xts[nt][:, k, :],
                            yt_sb[:, k, :],
                            start=(k == 0),
                            stop=(k == KD - 1))
                    ot = outp.tile([P, 512], F32)
                    if nt % 2 == 0:
                        nc.vector.tensor_copy(ot[:], po[:])
                    else:
                        nc.scalar.copy(ot[:], po[:])
                    nc.sync.dma_start(
                        o_d[nt * P:(nt + 1) * P, mc * 512:(mc + 1) * 512],
                        ot[:])

    nc.compile()
    return nc


_NC = None


def _get_nc():
    global _NC
    if _NC is None:
        _NC = build()
    return _NC


def kernel(input1, input2):
    global LAST_RESULT
    input1 = np.ascontiguousarray(np.asarray(input1, dtype=np.float32))
    input2 = np.ascontiguousarray(np.asarray(input2, dtype=np.float32))
    nc = _get_nc()
    in_maps = []
    for i in range(GRID_N):
        for j in range(GRID_M):
            in_maps.append({
                "x": input1[i * N_LOC:(i + 1) * N_LOC],
                "y": input2[j * M_LOC:(j + 1) * M_LOC],
            })
    res = run_bass_kernel_spmd(nc, in_maps, list(range(GRID_N * GRID_M)),
                               trace=TRACE)
    LAST_RESULT = res
    out = np.empty((N_FULL, M_FULL), dtype=np.float32)
    idx = 0
    for i in range(GRID_N):
        for j in range(GRID_M):
            out[i * N_LOC:(i + 1) * N_LOC,
                j * M_LOC:(j + 1) * M_LOC] = res.results[idx]["o"]
            idx += 1
    return out


# revision 12
# speedup vs baseline: 1.2088x; 1.0774x over previous
"""Pairwise cosine similarity [8192,1024]x[8192,1024] -> [8192,8192] on 8 trn2 cores.

Sharding: 4x2 grid. Core (i,j) takes input1 rows [2048*i, 2048*(i+1)) and
input2 rows [4096*j, 4096*(j+1)), computes its [2048, 4096] output block.
All cores run one SPMD program; the host slices inputs and assembles blocks.

Device program (per core):
  1. Normalize rows of x and y on-chip: ACT square w/ accum_out -> sqrt ->
     max(eps) -> reciprocal -> ACT copy w/ per-partition scale.
  2. PE transpose-mode (exact for fp32) moves D onto partitions:
     x^T [128, 8k, 2048], y^T chunks [128, 8k, 512].
  3. fp32r matmuls (1 cyc/row at N=512) accumulate 8 K-slabs into PSUM;
     DVE/ACT copy PSUM->SBUF; DMA out.
"""

import numpy as np

import concourse.bacc as bacc
import concourse.bass as bass
import concourse.masks as masks
import concourse.mybir as mybir
import concourse.tile as tile
from concourse.bass_utils import run_bass_kernel_spmd

P = 128
D = 1024
KD = D // P  # 8 k-slabs of the contraction dim
N_FULL = 8192
M_FULL = 8192
GRID_N, GRID_M = 4, 2
N_LOC = N_FULL // GRID_N  # 2048
M_LOC = M_FULL // GRID_M  # 4096
EPS = 1e-8
F32 = mybir.dt.float32
F32R = mybir.dt.float32r

# Set by test harness to capture profiling info; harness-default is off.
TRACE = False
LAST_RESULT = None


def build(n_loc=N_LOC, m_loc=M_LOC, n_cores=8):
    """Build + compile the SPMD program for one core's [n_loc, m_loc] block."""
    nt_tiles = n_loc // P
    yt_tiles = m_loc // P
    mc_chunks = m_loc // 512

    nc = bacc.Bacc("TRN2", target_bir_lowering=False, debug=False,
                   num_devices=n_cores)
    x_d = nc.dram_tensor("x", [n_loc, D], F32, kind="ExternalInput").ap()
    y_d = nc.dram_tensor("y", [m_loc, D], F32, kind="ExternalInput").ap()
    o_d = nc.dram_tensor("o", [n_loc, m_loc], F32, kind="ExternalOutput").ap()

    with tile.TileContext(nc) as tc:
        with (
            tc.tile_pool(name="persist", bufs=1) as persist,
            tc.tile_pool(name="stage", bufs=6) as stage,
            tc.tile_pool(name="sq", bufs=3) as sqp,
            tc.tile_pool(name="small", bufs=6) as small,
            tc.tile_pool(name="ytp", bufs=3) as ytp,
            tc.tile_pool(name="outp", bufs=4) as outp,
            tc.tile_pool(name="pst", bufs=2, space=bass.MemorySpace.PSUM) as pst,
            tc.tile_pool(name="pso", bufs=6, space=bass.MemorySpace.PSUM) as pso,
        ):
            ident = persist.tile([P, P], F32)
            masks.make_identity(nc, ident[:])
            # f32r identity so transposes run as fp32r (1.5 vs 2 cyc/row);
            # 0.0/1.0 are exact in fp32r so the transpose stays exact.
            ident_r = persist.tile([P, P], F32R)
            nc.vector.tensor_copy(ident_r[:], ident[:])
            # Per-n-tile x^T tiles (separate tags -> granular deps so the
            # first matmuls start before the whole X phase finishes).
            xts = [persist.tile([P, KD, P], F32R, name=f"xt{i}", tag=f"xt{i}")
                   for i in range(nt_tiles)]

            tile_seq = [0]

            def norm_transpose(src_rows, dst, dst_col0):
                # One [128, D] tile: load, normalize rows (rounding to f32r),
                # transpose the 8 [128,128] k-subtiles into
                # dst[:, k, dst_col0:dst_col0+128]. The square and scale
                # passes alternate between ACT and DVE to halve the prep
                # pipeline latency.
                use_dve = False  # bisect: DVE norm path suspected in HW crash
                tile_seq[0] += 1
                ts = stage.tile([P, D], F32)
                nc.sync.dma_start(ts[:], src_rows)
                sq = sqp.tile([P, D], F32)
                ss = small.tile([P, 1], F32)
                if use_dve:
                    nc.vector.tensor_tensor_reduce(
                        sq[:], ts[:], ts[:], 1.0, 0.0,
                        mybir.AluOpType.mult, mybir.AluOpType.add,
                        accum_out=ss[:])
                else:
                    nc.scalar.activation(sq[:], ts[:],
                                         mybir.ActivationFunctionType.Square,
                                         accum_out=ss[:])
                nrm = small.tile([P, 1], F32)
                nc.scalar.sqrt(nrm[:], ss[:])
                nc.vector.tensor_scalar_max(nrm[:], nrm[:], EPS)
                rinv = small.tile([P, 1], F32)
                nc.vector.reciprocal(rinv[:], nrm[:])
                tsr = sqp.tile([P, D], F32R, name="tsr", tag="tsr")
                if use_dve:
                    nc.vector.tensor_scalar_mul(tsr[:], ts[:], rinv[:])
                else:
                    nc.scalar.activation(tsr[:], ts[:],
                                         mybir.ActivationFunctionType.Copy,
                                         scale=rinv[:])
                for kg in range(KD // 4):
                    ps = pst.tile([P, 4, P], F32R)
                    for kk in range(4):
                        k = kg * 4 + kk
                        nc.tensor.transpose(ps[:, kk, :],
                                            tsr[:, k * P:(k + 1) * P],
                                            ident_r[:])
                    nc.vector.tensor_copy(
                        dst[:, kg * 4:(kg + 1) * 4, dst_col0:dst_col0 + P],
                        ps[:])

            yt_tiles = {}

            def prep_chunk(mc):
                yt_sb = ytp.tile([P, KD, 512], F32R, name=f"yt{mc}", tag="ytc")
                yt_tiles[mc] = yt_sb
                for v in range(4):
                    yt = mc * 4 + v
                    norm_transpose(y_d[yt * P:(yt + 1) * P, :], yt_sb, v * P)

            for mc in range(mc_chunks):
                if mc == 0:
                    prep_chunk(0)
                    if mc_chunks > 1:
                        prep_chunk(1)
                    for xt in range(nt_tiles):
                        norm_transpose(x_d[xt * P:(xt + 1) * P, :], xts[xt], 0)
                elif mc + 1 < mc_chunks:
                    prep_chunk(mc + 1)
                yt_sb = yt_tiles.pop(mc)
                for nt in range(nt_tiles):
                    po = pso.tile([P, 512], F32)
                    for k in range(KD):
                        nc.tensor.matmul(
                            po[:],
                            xts[nt][:, k, :],
                            yt_sb[:, k, :],
                            start=(k == 0),
                            stop=(k == KD - 1))
                    ot = outp.tile([P, 512], F32)
                    if nt % 2 == 0:
                        nc.vector.tensor_copy(ot[:], po[:])
                    else:
                        nc.scalar.copy(ot[:], po[:])
                    nc.sync.dma_start(
                        o_d[nt * P:(nt + 1) * P, mc * 512:(mc + 1) * 512],
                        ot[:])

    nc.compile()
    return nc


_NC = None


def _get_nc():
    global _NC
    if _NC is None:
        _NC = build()
    return _NC


def kernel(input1, input2):
    global LAST_RESULT
    input1 = np.ascontiguousarray(np.asarray(input1, dtype=np.float32))
    input2 = np.ascontiguousarray(np.asarray(input2, dtype=np.float32))
    nc = _get_nc()
    in_maps = []
    for i in range(GRID_N):
        for j in range(GRID_M):
            in_maps.append({
                "x": input1[i * N_LOC:(i + 1) * N_LOC],
                "y": input2[j * M_LOC:(j + 1) * M_LOC],
            })
    res = run_bass_kernel_spmd(nc, in_maps, list(range(GRID_N * GRID_M)),
                               trace=TRACE)
    LAST_RESULT = res
    out = np.empty((N_FULL, M_FULL), dtype=np.float32)
    idx = 0
    for i in range(GRID_N):
        for j in range(GRID_M):
            out[i * N_LOC:(i + 1) * N_LOC,
                j * M_LOC:(j + 1) * M_LOC] = res.results[idx]["o"]
            idx += 1
    return out
